# revision 27
# baseline (speedup 1.0000x reference)
"""Trainium2 Bass kernel for nn_AudioDeviceModel (dense_cnn, memory-bound).

The reference model applies a chain of dilated kernel-size-2 convs to a
length-1 sequence with SAME padding.  For dilation d the two taps land at
padded positions 0 and d while the real sample sits at position d//2, so
every conv after the first reduces to its bias; the first conv (dilation 1,
pad_low=0) reduces to tap 0: a dot product of x[b, :] with w1[0, :, 0].
The whole model is therefore

    out[b, j] = (x[b, :] . w1[0, :, 0]) * wd[0, j] + bd_eff[j]
    bd_eff[j] = (b1 + b2 + b3 + b4 + b5) * wd[0, j] + bd[j]

(verified numerically against the jax reference to 1e-7).  This is a pure
memory-bound row-wise dot product over a 512 MiB matrix.

Strategy: data-parallel across 8 NeuronCores (1024 rows each).  The
per-core DMA fabric is 16 engines x ~27 GB/s = ~433 GB/s, so the 64 MiB
x-shard floors at ~155 us of bus time; everything else must hide under
that stream.  Measured facts that shaped this design (see traces):
  - DVE fp32 multiply-reduce runs at 1.061 ns/elem (142 us total here), so
    the DVE must run ONLY the streaming passes and nothing else.
  - The HW DGE rings start moving data ~9 us into the kernel; SWDGE
    (gpsimd) starts LATER (~12 us) and is slower — useless for prefetch.
  - Small x tiles (<= 2 MiB) collapse the pipeline when the x pool slots
    fill before the DVE starts: every DMA enqueue then waits on an STT
    `bufs` tiles back and the fixed ~2.5 us/tile latency chain enters the
    steady state (-25% throughput).  Big 3-4 MiB tiles amortize it.
  - The first STT needs chunk0's v replicated + the first x tile; both
    cross the same 433 GB/s pipe, so chunk 0 must be narrow.

Design: column phases (2048, 8192, 6144) x 8 row-blocks of 128:
  - phase 0 narrow: v0 (1 MiB, stride-0 DMA broadcast split across both
    rings) + 1 MiB x tiles -> first STT at ~17 us.
  - phases 1-2 wide (4/3 MiB tiles, 3-slot pool = ~29 us runway): DVE has
    a 4-7% per-tile margin over DMA, so the backlog drains and DMA paces.
  - v for phases 1-2 replicated on-chip (ones[1,128].T @ v on the idle PE,
    K=1 so the product is exact; PSUM->SBUF copies on the idle Activation
    engine), saving 3.5 MiB of bus traffic vs a DMA broadcast.
  - epilogue off the DVE: Activation reduces acc via its accumulator and
    forms t*wd via its per-partition scale; Pool adds bd_eff and writes
    out (the last block uses DVE+SP — faster tail).
  - the very last x tile is split in two so the final STT tail is ~3 us.

This container's walrus build only accepts ONE on_wait and ONE on_update
per instruction, while Tile emits multi-wait instructions (kernel-tail
drain, multi-dependency compute ops).  legalize_bir_sync() splits the
extras into standalone EventSemaphore/NoOp instructions on the same engine
(sequencers are in-order, so a wait immediately before an instruction is
equivalent; trailing updates only on non-DMA instructions).
"""

import json

import numpy as np

import concourse.bass as bass
import concourse.mybir as mybir
import concourse.tile as tile
from concourse.bass_utils import run_bass_kernel_spmd

FP32 = mybir.dt.float32

N_CORES = 8
B_FULL = 8192
L = 16384
J = 128
B_CORE = B_FULL // N_CORES  # 1024
P = 128                     # SBUF partitions
N_BB = B_CORE // P          # 8 row-blocks per core

CHUNKS = (2048, 8192, 6144)         # column phases; sum == L
MM = 512                            # PE broadcast width (one PSUM bank)
VR_PIECE = 4096                     # vrow staging piece (SBUF address space)


def legalize_bir_sync(bir_bytes: bytes) -> bytes:
    """Split >1 on_wait / on_update per instruction for this walrus build."""
    mod = json.loads(bir_bytes)
    for fn in mod["functions"]:
        for bb in fn["blocks"]:
            out = []
            for ins in bb["instructions"]:
                si = ins.get("sync_info")
                waits = (si or {}).get("on_wait") or []
                ups = (si or {}).get("on_update") or []
                if len(waits) > 1:
                    for i, w in enumerate(waits[:-1]):
                        out.append({
                            "debug": ins.get("debug"),
                            "engine": ins["engine"],
                            "ins": [],
                            "outs": [],
                            "name": f"{ins['name']}_lw{i}",
                            "opcode": "EventSemaphore",
                            "sync_info": {"on_update": [], "on_wait": [w]},
                        })
                    si["on_wait"] = [waits[-1]]
                out.append(ins)
                if len(ups) > 1:
                    if ins.get("opcode") == "DMACopy":
                        raise RuntimeError(
                            f"multi-update on DMA {ins['name']} cannot be legalized"
                        )
                    for i, u in enumerate(ups[1:]):
                        out.append({
                            "debug": ins.get("debug"),
                            "engine": ins["engine"],
                            "ins": [],
                            "outs": [],
                            "name": f"{ins['name']}_lu{i}",
                            "opcode": "NoOp",
                            "sync_info": {"on_update": [u], "on_wait": []},
                        })
                    si["on_update"] = [ups[0]]
            bb["instructions"] = out
    return json.dumps(mod).encode()


def install_legalizer(nc):
    orig = nc.to_json_bytes

    def patched():
        return legalize_bir_sync(orig())

    nc.to_json_bytes = patched
    return nc


def build_module() -> bass.Bass:
    n_ch = len(CHUNKS)
    offs = [sum(CHUNKS[:i]) for i in range(n_ch)]
    c0 = CHUNKS[0]
    nc = bass.Bass()
    x_ds = [
        nc.dram_tensor(f"x{bb}", [P, L], FP32, kind="ExternalInput")
        for bb in range(N_BB)
    ]
    v_d = nc.dram_tensor("v", [L], FP32, kind="ExternalInput")
    wd_d = nc.dram_tensor("wdrow", [J], FP32, kind="ExternalInput")
    bd_d = nc.dram_tensor("bdeff", [J], FP32, kind="ExternalInput")
    out_d = nc.dram_tensor("out", [B_CORE, J], FP32, kind="ExternalOutput")

    rings = None  # set below

    with tile.TileContext(nc) as tc:
        with (
            tc.tile_pool(name="consts", bufs=1) as consts,
            tc.tile_pool(name="xp", bufs=3) as xp,
            tc.tile_pool(name="vrp", bufs=1) as vrp,
            tc.tile_pool(name="accp", bufs=2) as accp,
            tc.tile_pool(name="outp", bufs=2) as outp,
            tc.tile_pool(name="psum", bufs=8, space="PSUM") as psum,
        ):
            rings = (nc.sync, nc.scalar)

            # Tiny consts on the gpsimd (SWDGE) ring.
            wd_b = consts.tile([P, J], FP32)
            nc.gpsimd.dma_start(out=wd_b, in_=wd_d[:].unsqueeze(0).partition_broadcast(P))
            bd_b = consts.tile([P, J], FP32)
            nc.gpsimd.dma_start(out=bd_b, in_=bd_d[:].unsqueeze(0).partition_broadcast(P))
            ones = consts.tile([1, P], FP32)
            nc.gpsimd.memset(ones, 1.0)

            # v replicated across partitions, one tile per chunk so each
            # STT depends only on ITS chunk's writers (no false deps).
            v_cs = [
                consts.tile([P, CHUNKS[c]], FP32, name=f"vc{c}", tag=f"vc{c}")
                for c in range(n_ch)
            ]
            # chunk 0: stride-0 DMA broadcast (4 MiB of bus traffic) split
            # across both rings ahead of their phase-0 x tiles.  This costs
            # ~9us of bus time but keeps phase 0's DVE demand well under
            # its DMA time, which is what keeps the pipeline out of the
            # slot-gated regime (measured: SWDGE starts LATER than the HW
            # rings, so prefetching via gpsimd does not work).
            h = c0 // 2
            for r in range(2):
                rings[r].dma_start(
                    out=v_cs[0][:, r * h:(r + 1) * h],
                    in_=v_d[r * h:(r + 1) * h].unsqueeze(0).partition_broadcast(P),
                )

            def emit_vchunk(c: int):
                # chunk c (c>=1): on-chip replicate.  ones[1,P].T @ v
                # (K=1 so each output is a single product => exact copy);
                # PSUM->SBUF copies on the Activation engine.  vrow is
                # staged through SBUF in <=VR_PIECE sub-pieces.
                f, off = CHUNKS[c], offs[c]
                for sub in range(0, f, VR_PIECE):
                    fs = min(VR_PIECE, f - sub)
                    vr_t = vrp.tile([1, fs], FP32, name=f"vr{c}_{sub}", tag="vr")
                    nc.gpsimd.dma_start(
                        out=vr_t, in_=v_d[off + sub:off + sub + fs].unsqueeze(0)
                    )
                    for k in range(fs // MM):
                        pt = psum.tile([P, MM], FP32, name=f"pt{c}_{sub}_{k}", tag="pt")
                        nc.tensor.matmul(
                            pt, ones, vr_t[:, k * MM:(k + 1) * MM],
                            start=True, stop=True,
                        )
                        nc.scalar.copy(
                            out=v_cs[c][:, sub + k * MM:sub + (k + 1) * MM], in_=pt
                        )

            # block 7's final tile is split in half to shorten the kernel
            # tail, so its accumulator gets one extra column.
            accs = [
                accp.tile(
                    [P, n_ch + (1 if bb == N_BB - 1 else 0)], FP32,
                    name=f"acc{bb}", tag=f"acc{bb}",
                )
                for bb in range(N_BB)
            ]

            for c in range(n_ch):
                f, off = CHUNKS[c], offs[c]
                # x DMAs for this phase first so both rings stay fed...
                last = c == n_ch - 1
                xts = []
                for bb in range(N_BB):
                    if last and bb == N_BB - 1:
                        # split the kernel's very last tile in two so the
                        # final STT (the tail) is half as long
                        h2 = f // 2
                        pair = []
                        for s in range(2):
                            x_t = xp.tile([P, h2], FP32, name=f"x{c}_{bb}_{s}", tag="x")
                            rings[bb % 2].dma_start(
                                out=x_t,
                                in_=x_ds[bb][:, off + s * h2:off + (s + 1) * h2],
                            )
                            pair.append(x_t)
                        xts.append(pair)
                        continue
                    x_t = xp.tile([P, f], FP32, name=f"x{c}_{bb}", tag="x")
                    rings[bb % 2].dma_start(out=x_t, in_=x_ds[bb][:, off:off + f])
                    xts.append(x_t)
                # ...then the NEXT phase's v replication (Act engine work
                # lands between this phase's and next phase's enqueues).
                if c + 1 < n_ch:
                    emit_vchunk(c + 1)
                for bb in range(N_BB):
                    # x_t *= v (in place); acc[:, c] = sum over free dim.
                    # The DVE runs ONLY these streaming passes.
                    if last and bb == N_BB - 1:
                        h2 = f // 2
                        for s in range(2):
                            nc.vector.scalar_tensor_tensor(
                                out=xts[bb][s],
                                in0=xts[bb][s],
                                scalar=1.0,
                                in1=v_cs[c][:, s * h2:(s + 1) * h2],
                                op0=mybir.AluOpType.mult,
                                op1=mybir.AluOpType.mult,
                                accum_out=accs[bb][:, c + s:c + s + 1],
                            )
                    else:
                        nc.vector.scalar_tensor_tensor(
                            out=xts[bb],
                            in0=xts[bb],
                            scalar=1.0,
                            in1=v_cs[c],
                            op0=mybir.AluOpType.mult,
                            op1=mybir.AluOpType.mult,
                            accum_out=accs[bb][:, c:c + 1],
                        )
                    if c == n_ch - 1:
                        # epilogue off the DVE: Act reduces acc via
                        # activation's accumulator and forms t*wd via the
                        # per-partition scale operand; Pool adds bd_eff and
                        # writes out.
                        tacc = accp.tile(
                            [P, accs[bb].shape[1]], FP32, name=f"ta{bb}", tag="ta"
                        )
                        t = accp.tile([P, 1], FP32, name=f"t{bb}", tag="t")
                        nc.scalar.activation(
                            out=tacc, in_=accs[bb],
                            func=mybir.ActivationFunctionType.Copy,
                            bias=0.0, scale=1.0, accum_out=t,
                        )
                        o1 = outp.tile([P, J], FP32, name=f"o1_{bb}", tag="o1")
                        nc.scalar.activation(
                            out=o1, in_=wd_b,
                            func=mybir.ActivationFunctionType.Copy,
                            bias=0.0, scale=t,
                        )
                        o_t = outp.tile([P, J], FP32, name=f"o{bb}", tag="o")
                        if bb == N_BB - 1:
                            # last block is the kernel tail: Pool's
                            # tensor_add is ~2.5us, DVE's is ~0.3us, and
                            # the SP ring enqueues faster than SWDGE.
                            nc.vector.tensor_add(out=o_t, in0=o1, in1=bd_b)
                            nc.sync.dma_start(
                                out=out_d[bb * P:(bb + 1) * P, :], in_=o_t
                            )
                        else:
                            nc.gpsimd.tensor_add(out=o_t, in0=o1, in1=bd_b)
                            nc.gpsimd.dma_start(
                                out=out_d[bb * P:(bb + 1) * P, :], in_=o_t
                            )
    install_legalizer(nc)
    return nc


_module_cache: dict = {}


def get_module() -> bass.Bass:
    if "nc" not in _module_cache:
        _module_cache["nc"] = build_module()
    return _module_cache["nc"]


def make_in_maps(inputs: dict) -> list[dict]:
    """Shard the full inputs into one input map per core (pure data parallel
    on the batch dim; tiny weights replicated)."""
    x = np.ascontiguousarray(np.asarray(inputs["x"], dtype=np.float32))
    w1 = np.asarray(inputs["w1"], dtype=np.float32)
    v = np.ascontiguousarray(w1[0, :, 0])
    s0 = float(sum(
        np.asarray(inputs[k], np.float32).reshape(-1)[0]
        for k in ("b1", "b2", "b3", "b4", "b5")
    ))
    wd_row = np.ascontiguousarray(np.asarray(inputs["wd"], np.float32)[0, :])
    bd = np.asarray(inputs["bd"], np.float32).reshape(-1)
    bd_eff = np.ascontiguousarray((s0 * wd_row + bd).astype(np.float32))

    maps = []
    for c in range(N_CORES):
        m = {"v": v, "wdrow": wd_row, "bdeff": bd_eff}
        base = c * B_CORE
        for bb in range(B_CORE // P):
            m[f"x{bb}"] = np.ascontiguousarray(x[base + bb * P:base + (bb + 1) * P])
        maps.append(m)
    return maps


def kernel(**inputs) -> np.ndarray:
    nc = get_module()
    in_maps = make_in_maps(inputs)
    res = run_bass_kernel_spmd(nc, in_maps, core_ids=list(range(N_CORES)))
    return np.concatenate([r["out"] for r in res.results], axis=0)


# revision 29
# speedup vs baseline: 1.0409x; 1.0409x over previous
"""Trainium2 Bass kernel for nn_AudioDeviceModel (dense_cnn, memory-bound).

The reference model applies a chain of dilated kernel-size-2 convs to a
length-1 sequence with SAME padding.  For dilation d the two taps land at
padded positions 0 and d while the real sample sits at position d//2, so
every conv after the first reduces to its bias; the first conv (dilation 1,
pad_low=0) reduces to tap 0: a dot product of x[b, :] with w1[0, :, 0].
The whole model is therefore

    out[b, j] = (x[b, :] . w1[0, :, 0]) * wd[0, j] + bd_eff[j]
    bd_eff[j] = (b1 + b2 + b3 + b4 + b5) * wd[0, j] + bd[j]

(verified numerically against the jax reference to 1e-7).  This is a pure
memory-bound row-wise dot product over a 512 MiB matrix.

Strategy: data-parallel across 8 NeuronCores (1024 rows each).  The
per-core DMA fabric is 16 engines x ~27 GB/s = ~433 GB/s, so the 64 MiB
x-shard floors at ~155 us of bus time; everything else must hide under
that stream.  Measured facts that shaped this design (see traces):
  - DVE fp32 multiply-reduce runs at 1.061 ns/elem (142 us total here), so
    the DVE must run ONLY the streaming passes and nothing else.
  - The HW DGE rings start moving data ~9 us into the kernel; SWDGE
    (gpsimd) starts LATER (~12 us) and is slower — useless for prefetch.
  - Small x tiles (<= 2 MiB) collapse the pipeline when the x pool slots
    fill before the DVE starts: every DMA enqueue then waits on an STT
    `bufs` tiles back and the fixed ~2.5 us/tile latency chain enters the
    steady state (-25% throughput).  Big 3-4 MiB tiles amortize it.
  - The first STT needs chunk0's v replicated + the first x tile; both
    cross the same 433 GB/s pipe, so chunk 0 must be narrow.

Design: column phases (2048, 8192, 6144) x 8 row-blocks of 128:
  - phase 0 narrow: v0 (1 MiB, stride-0 DMA broadcast split across both
    rings) + 1 MiB x tiles -> first STT at ~17 us.
  - phases 1-2 wide (4/3 MiB tiles, 3-slot pool = ~29 us runway): DVE has
    a 4-7% per-tile margin over DMA, so the backlog drains and DMA paces.
  - v for phases 1-2 replicated on-chip (ones[1,128].T @ v on the idle PE,
    K=1 so the product is exact; PSUM->SBUF copies on the idle Activation
    engine), saving 3.5 MiB of bus traffic vs a DMA broadcast.
  - epilogue off the DVE: Activation reduces acc via its accumulator and
    forms t*wd via its per-partition scale; Pool adds bd_eff and writes
    out (the last block uses DVE+SP — faster tail).
  - the very last x tile is split in two so the final STT tail is ~3 us.

This container's walrus build only accepts ONE on_wait and ONE on_update
per instruction, while Tile emits multi-wait instructions (kernel-tail
drain, multi-dependency compute ops).  legalize_bir_sync() splits the
extras into standalone EventSemaphore/NoOp instructions on the same engine
(sequencers are in-order, so a wait immediately before an instruction is
equivalent; trailing updates only on non-DMA instructions).
"""

import json

import numpy as np

import concourse.bass as bass
import concourse.mybir as mybir
import concourse.tile as tile
from concourse.bass_utils import run_bass_kernel_spmd

FP32 = mybir.dt.float32

N_CORES = 8
B_FULL = 8192
L = 16384
J = 128
B_CORE = B_FULL // N_CORES  # 1024
P = 128                     # SBUF partitions
N_BB = B_CORE // P          # 8 row-blocks per core

CHUNKS = (8192, 8192)               # column phases; sum == L
MM = 512                            # PE broadcast width (one PSUM bank)
VR_PIECE = 4096                     # vrow staging piece (SBUF address space)


def legalize_bir_sync(bir_bytes: bytes) -> bytes:
    """Split >1 on_wait / on_update per instruction for this walrus build."""
    mod = json.loads(bir_bytes)
    for fn in mod["functions"]:
        for bb in fn["blocks"]:
            out = []
            for ins in bb["instructions"]:
                si = ins.get("sync_info")
                waits = (si or {}).get("on_wait") or []
                ups = (si or {}).get("on_update") or []
                if len(waits) > 1:
                    for i, w in enumerate(waits[:-1]):
                        out.append({
                            "debug": ins.get("debug"),
                            "engine": ins["engine"],
                            "ins": [],
                            "outs": [],
                            "name": f"{ins['name']}_lw{i}",
                            "opcode": "EventSemaphore",
                            "sync_info": {"on_update": [], "on_wait": [w]},
                        })
                    si["on_wait"] = [waits[-1]]
                out.append(ins)
                if len(ups) > 1:
                    if ins.get("opcode") == "DMACopy":
                        raise RuntimeError(
                            f"multi-update on DMA {ins['name']} cannot be legalized"
                        )
                    for i, u in enumerate(ups[1:]):
                        out.append({
                            "debug": ins.get("debug"),
                            "engine": ins["engine"],
                            "ins": [],
                            "outs": [],
                            "name": f"{ins['name']}_lu{i}",
                            "opcode": "NoOp",
                            "sync_info": {"on_update": [u], "on_wait": []},
                        })
                    si["on_update"] = [ups[0]]
            bb["instructions"] = out
    return json.dumps(mod).encode()


def install_legalizer(nc):
    orig = nc.to_json_bytes

    def patched():
        return legalize_bir_sync(orig())

    nc.to_json_bytes = patched
    return nc


def build_module() -> bass.Bass:
    n_ch = len(CHUNKS)
    offs = [sum(CHUNKS[:i]) for i in range(n_ch)]
    c0 = CHUNKS[0]
    nc = bass.Bass()
    x_ds = [
        nc.dram_tensor(f"x{bb}", [P, L], FP32, kind="ExternalInput")
        for bb in range(N_BB)
    ]
    v_d = nc.dram_tensor("v", [L], FP32, kind="ExternalInput")
    wd_d = nc.dram_tensor("wdrow", [J], FP32, kind="ExternalInput")
    bd_d = nc.dram_tensor("bdeff", [J], FP32, kind="ExternalInput")
    out_d = nc.dram_tensor("out", [B_CORE, J], FP32, kind="ExternalOutput")

    rings = None  # set below

    with tile.TileContext(nc) as tc:
        with (
            tc.tile_pool(name="consts", bufs=1) as consts,
            tc.tile_pool(name="xp", bufs=3) as xp,
            tc.tile_pool(name="vrp", bufs=1) as vrp,
            tc.tile_pool(name="accp", bufs=2) as accp,
            tc.tile_pool(name="outp", bufs=2) as outp,
            tc.tile_pool(name="psum", bufs=8, space="PSUM") as psum,
        ):
            rings = (nc.sync, nc.scalar)

            # Tiny consts on the gpsimd (SWDGE) ring.
            wd_b = consts.tile([P, J], FP32)
            nc.gpsimd.dma_start(out=wd_b, in_=wd_d[:].unsqueeze(0).partition_broadcast(P))
            bd_b = consts.tile([P, J], FP32)
            nc.gpsimd.dma_start(out=bd_b, in_=bd_d[:].unsqueeze(0).partition_broadcast(P))
            ones = consts.tile([1, P], FP32)
            nc.gpsimd.memset(ones, 1.0)

            # v replicated across partitions, one tile per chunk so each
            # STT depends only on ITS chunk's writers (no false deps).
            v_cs = [
                consts.tile([P, CHUNKS[c]], FP32, name=f"vc{c}", tag=f"vc{c}")
                for c in range(n_ch)
            ]
            # chunk 0: stride-0 DMA broadcast (4 MiB of bus traffic) split
            # across both rings ahead of their phase-0 x tiles.  This costs
            # ~9us of bus time but keeps phase 0's DVE demand well under
            # its DMA time, which is what keeps the pipeline out of the
            # slot-gated regime (measured: SWDGE starts LATER than the HW
            # rings, so prefetching via gpsimd does not work).
            h = c0 // 2
            for r in range(2):
                rings[r].dma_start(
                    out=v_cs[0][:, r * h:(r + 1) * h],
                    in_=v_d[r * h:(r + 1) * h].unsqueeze(0).partition_broadcast(P),
                )

            def emit_vchunk(c: int):
                # chunk c (c>=1): on-chip replicate.  ones[1,P].T @ v
                # (K=1 so each output is a single product => exact copy);
                # PSUM->SBUF copies on the Activation engine.  vrow is
                # staged through SBUF in <=VR_PIECE sub-pieces.
                f, off = CHUNKS[c], offs[c]
                for sub in range(0, f, VR_PIECE):
                    fs = min(VR_PIECE, f - sub)
                    vr_t = vrp.tile([1, fs], FP32, name=f"vr{c}_{sub}", tag="vr")
                    nc.gpsimd.dma_start(
                        out=vr_t, in_=v_d[off + sub:off + sub + fs].unsqueeze(0)
                    )
                    for k in range(fs // MM):
                        pt = psum.tile([P, MM], FP32, name=f"pt{c}_{sub}_{k}", tag="pt")
                        nc.tensor.matmul(
                            pt, ones, vr_t[:, k * MM:(k + 1) * MM],
                            start=True, stop=True,
                        )
                        nc.scalar.copy(
                            out=v_cs[c][:, sub + k * MM:sub + (k + 1) * MM], in_=pt
                        )

            # block 7's final tile is split in half to shorten the kernel
            # tail, so its accumulator gets one extra column.
            accs = [
                accp.tile(
                    [P, n_ch + (1 if bb == N_BB - 1 else 0)], FP32,
                    name=f"acc{bb}", tag=f"acc{bb}",
                )
                for bb in range(N_BB)
            ]

            for c in range(n_ch):
                f, off = CHUNKS[c], offs[c]
                # x DMAs for this phase first so both rings stay fed...
                last = c == n_ch - 1

                def dual_ring_dma(x_t, bb, lo, hi):
                    # Every x tile is fetched by BOTH rings (half-columns
                    # each).  The rings then carry every tile in lockstep:
                    # per-tile arrival latency is halved (smaller head) and
                    # neither ring ever idles while the other streams.
                    mid = (lo + hi) // 2
                    rings[0].dma_start(
                        out=x_t[:, 0:mid - lo], in_=x_ds[bb][:, lo:mid]
                    )
                    rings[1].dma_start(
                        out=x_t[:, mid - lo:hi - lo], in_=x_ds[bb][:, mid:hi]
                    )

                xts = []
                for bb in range(N_BB):
                    if last and bb == N_BB - 1:
                        # split the kernel's very last tile in two so the
                        # final STT (the tail) is half as long
                        h2 = f // 2
                        pair = []
                        for s in range(2):
                            x_t = xp.tile([P, h2], FP32, name=f"x{c}_{bb}_{s}", tag="x")
                            dual_ring_dma(x_t, bb, off + s * h2, off + (s + 1) * h2)
                            pair.append(x_t)
                        xts.append(pair)
                        continue
                    x_t = xp.tile([P, f], FP32, name=f"x{c}_{bb}", tag="x")
                    dual_ring_dma(x_t, bb, off, off + f)
                    xts.append(x_t)
                # ...then the NEXT phase's v replication (Act engine work
                # lands between this phase's and next phase's enqueues).
                if c + 1 < n_ch:
                    emit_vchunk(c + 1)
                for bb in range(N_BB):
                    # x_t *= v (in place); acc[:, c] = sum over free dim.
                    # The DVE runs ONLY these streaming passes.
                    if last and bb == N_BB - 1:
                        h2 = f // 2
                        for s in range(2):
                            nc.vector.scalar_tensor_tensor(
                                out=xts[bb][s],
                                in0=xts[bb][s],
                                scalar=1.0,
                                in1=v_cs[c][:, s * h2:(s + 1) * h2],
                                op0=mybir.AluOpType.mult,
                                op1=mybir.AluOpType.mult,
                                accum_out=accs[bb][:, c + s:c + s + 1],
                            )
                    else:
                        nc.vector.scalar_tensor_tensor(
                            out=xts[bb],
                            in0=xts[bb],
                            scalar=1.0,
                            in1=v_cs[c],
                            op0=mybir.AluOpType.mult,
                            op1=mybir.AluOpType.mult,
                            accum_out=accs[bb][:, c:c + 1],
                        )
                    if c == n_ch - 1:
                        # epilogue off the DVE: Act reduces acc via
                        # activation's accumulator and forms t*wd via the
                        # per-partition scale operand; Pool adds bd_eff and
                        # writes out.
                        tacc = accp.tile(
                            [P, accs[bb].shape[1]], FP32, name=f"ta{bb}", tag="ta"
                        )
                        t = accp.tile([P, 1], FP32, name=f"t{bb}", tag="t")
                        nc.scalar.activation(
                            out=tacc, in_=accs[bb],
                            func=mybir.ActivationFunctionType.Copy,
                            bias=0.0, scale=1.0, accum_out=t,
                        )
                        o1 = outp.tile([P, J], FP32, name=f"o1_{bb}", tag="o1")
                        nc.scalar.activation(
                            out=o1, in_=wd_b,
                            func=mybir.ActivationFunctionType.Copy,
                            bias=0.0, scale=t,
                        )
                        o_t = outp.tile([P, J], FP32, name=f"o{bb}", tag="o")
                        if bb == N_BB - 1:
                            # last block is the kernel tail: Pool's
                            # tensor_add is ~2.5us, DVE's is ~0.3us, and
                            # the SP ring enqueues faster than SWDGE.
                            nc.vector.tensor_add(out=o_t, in0=o1, in1=bd_b)
                            nc.sync.dma_start(
                                out=out_d[bb * P:(bb + 1) * P, :], in_=o_t
                            )
                        else:
                            nc.gpsimd.tensor_add(out=o_t, in0=o1, in1=bd_b)
                            nc.gpsimd.dma_start(
                                out=out_d[bb * P:(bb + 1) * P, :], in_=o_t
                            )
    install_legalizer(nc)
    return nc


_module_cache: dict = {}


def get_module() -> bass.Bass:
    if "nc" not in _module_cache:
        _module_cache["nc"] = build_module()
    return _module_cache["nc"]


def make_in_maps(inputs: dict) -> list[dict]:
    """Shard the full inputs into one input map per core (pure data parallel
    on the batch dim; tiny weights replicated)."""
    x = np.ascontiguousarray(np.asarray(inputs["x"], dtype=np.float32))
    w1 = np.asarray(inputs["w1"], dtype=np.float32)
    v = np.ascontiguousarray(w1[0, :, 0])
    s0 = float(sum(
        np.asarray(inputs[k], np.float32).reshape(-1)[0]
        for k in ("b1", "b2", "b3", "b4", "b5")
    ))
    wd_row = np.ascontiguousarray(np.asarray(inputs["wd"], np.float32)[0, :])
    bd = np.asarray(inputs["bd"], np.float32).reshape(-1)
    bd_eff = np.ascontiguousarray((s0 * wd_row + bd).astype(np.float32))

    maps = []
    for c in range(N_CORES):
        m = {"v": v, "wdrow": wd_row, "bdeff": bd_eff}
        base = c * B_CORE
        for bb in range(B_CORE // P):
            m[f"x{bb}"] = np.ascontiguousarray(x[base + bb * P:base + (bb + 1) * P])
        maps.append(m)
    return maps


def kernel(**inputs) -> np.ndarray:
    nc = get_module()
    in_maps = make_in_maps(inputs)
    res = run_bass_kernel_spmd(nc, in_maps, core_ids=list(range(N_CORES)))
    return np.concatenate([r["out"] for r in res.results], axis=0)


# revision 33
# speedup vs baseline: 1.1728x; 1.1267x over previous
"""Trainium2 Bass kernel for nn_AudioDeviceModel (dense_cnn, memory-bound).

The reference model applies a chain of dilated kernel-size-2 convs to a
length-1 sequence with SAME padding.  For dilation d the two taps land at
padded positions 0 and d while the real sample sits at position d//2, so
every conv after the first reduces to its bias; the first conv (dilation 1,
pad_low=0) reduces to tap 0: a dot product of x[b, :] with w1[0, :, 0].
The whole model is therefore

    out[b, j] = (x[b, :] . w1[0, :, 0]) * wd[0, j] + bd_eff[j]
    bd_eff[j] = (b1 + b2 + b3 + b4 + b5) * wd[0, j] + bd[j]

(verified numerically against the jax reference to 1e-7).  This is a pure
memory-bound row-wise dot product over a 512 MiB matrix.

Strategy: data-parallel across 8 NeuronCores (1024 rows each).  The
per-core DMA fabric is 16 engines x ~27 GB/s = ~433 GB/s, so the 64 MiB
x-shard floors at ~155 us of bus time; everything else must hide under
that stream.  Measured facts that shaped this design (see traces):
  - DVE fp32 multiply-reduce runs at 1.061 ns/elem (142 us total here), so
    the DVE must run ONLY the streaming passes and nothing else.
  - The HW DGE rings start moving data ~9 us into the kernel; SWDGE
    (gpsimd) starts LATER (~12 us) and is slower — useless for prefetch.
  - Small x tiles (<= 2 MiB) collapse the pipeline when the x pool slots
    fill before the DVE starts: every DMA enqueue then waits on an STT
    `bufs` tiles back and the fixed ~2.5 us/tile latency chain enters the
    steady state (-25% throughput).  Big 3-4 MiB tiles amortize it.
  - The first STT needs chunk0's v replicated + the first x tile; both
    cross the same 433 GB/s pipe, so chunk 0 must be narrow.

Design: two uniform 8192-wide column phases x 8 row-blocks of 128 (16
x tiles of 4 MiB, 3-slot pool).  Uniform big tiles are what keep the
pipeline stable: once slot-gated, a ring's solo 4 MiB transfer still
beats three STT periods, so the DVE never starves (measured: every
mixed-size or narrow-tile variant regressed 10-17%).
  - chunk-0 v (4 MiB) via stride-0 DMA broadcast split across both rings
    ahead of their phase-0 x tiles.
  - chunk-1 v replicated on-chip (ones[1,128].T @ v on the idle PE, K=1
    so the product is exact; PSUM->SBUF copies on the idle Activation
    engine), saving 4 MiB of bus traffic vs a DMA broadcast.
  - epilogue off the DVE: Activation reduces acc via its accumulator and
    forms t*wd via its per-partition scale; Pool adds bd_eff and writes
    out (the last block uses DVE+SP — faster tail).

This container's walrus build only accepts ONE on_wait and ONE on_update
per instruction, while Tile emits multi-wait instructions (kernel-tail
drain, multi-dependency compute ops).  legalize_bir_sync() splits the
extras into standalone EventSemaphore/NoOp instructions on the same engine
(sequencers are in-order, so a wait immediately before an instruction is
equivalent; trailing updates only on non-DMA instructions).
"""

import json

import numpy as np

import concourse.bass as bass
import concourse.mybir as mybir
import concourse.tile as tile
from concourse.bass_utils import run_bass_kernel_spmd

FP32 = mybir.dt.float32

N_CORES = 8
B_FULL = 8192
L = 16384
J = 128
B_CORE = B_FULL // N_CORES  # 1024
P = 128                     # SBUF partitions
N_BB = B_CORE // P          # 8 row-blocks per core

CHUNKS = (8192, 8192)               # column phases; sum == L
MM = 512                            # PE broadcast width (one PSUM bank)
VR_PIECE = 4096                     # vrow staging piece (SBUF address space)


def legalize_bir_sync(bir_bytes: bytes) -> bytes:
    """Split >1 on_wait / on_update per instruction for this walrus build."""
    mod = json.loads(bir_bytes)
    for fn in mod["functions"]:
        for bb in fn["blocks"]:
            out = []
            for ins in bb["instructions"]:
                si = ins.get("sync_info")
                waits = (si or {}).get("on_wait") or []
                ups = (si or {}).get("on_update") or []
                if len(waits) > 1:
                    for i, w in enumerate(waits[:-1]):
                        out.append({
                            "debug": ins.get("debug"),
                            "engine": ins["engine"],
                            "ins": [],
                            "outs": [],
                            "name": f"{ins['name']}_lw{i}",
                            "opcode": "EventSemaphore",
                            "sync_info": {"on_update": [], "on_wait": [w]},
                        })
                    si["on_wait"] = [waits[-1]]
                out.append(ins)
                if len(ups) > 1:
                    if ins.get("opcode") == "DMACopy":
                        raise RuntimeError(
                            f"multi-update on DMA {ins['name']} cannot be legalized"
                        )
                    for i, u in enumerate(ups[1:]):
                        out.append({
                            "debug": ins.get("debug"),
                            "engine": ins["engine"],
                            "ins": [],
                            "outs": [],
                            "name": f"{ins['name']}_lu{i}",
                            "opcode": "NoOp",
                            "sync_info": {"on_update": [u], "on_wait": []},
                        })
                    si["on_update"] = [ups[0]]
            bb["instructions"] = out
    return json.dumps(mod).encode()


def install_legalizer(nc):
    orig = nc.to_json_bytes

    def patched():
        return legalize_bir_sync(orig())

    nc.to_json_bytes = patched
    return nc


def build_module() -> bass.Bass:
    n_ch = len(CHUNKS)
    offs = [sum(CHUNKS[:i]) for i in range(n_ch)]
    c0 = CHUNKS[0]
    nc = bass.Bass()
    x_ds = [
        nc.dram_tensor(f"x{bb}", [P, L], FP32, kind="ExternalInput")
        for bb in range(N_BB)
    ]
    v_d = nc.dram_tensor("v", [L], FP32, kind="ExternalInput")
    wd_d = nc.dram_tensor("wdrow", [J], FP32, kind="ExternalInput")
    bd_d = nc.dram_tensor("bdeff", [J], FP32, kind="ExternalInput")
    out_d = nc.dram_tensor("out", [B_CORE, J], FP32, kind="ExternalOutput")

    rings = None  # set below

    with tile.TileContext(nc) as tc:
        with (
            tc.tile_pool(name="consts", bufs=1) as consts,
            tc.tile_pool(name="xp", bufs=3) as xp,
            tc.tile_pool(name="vrp", bufs=1) as vrp,
            tc.tile_pool(name="accp", bufs=2) as accp,
            tc.tile_pool(name="outp", bufs=2) as outp,
            tc.tile_pool(name="psum", bufs=8, space="PSUM") as psum,
        ):
            rings = (nc.sync, nc.scalar)

            # Tiny consts on the gpsimd (SWDGE) ring.
            wd_b = consts.tile([P, J], FP32)
            nc.gpsimd.dma_start(out=wd_b, in_=wd_d[:].unsqueeze(0).partition_broadcast(P))
            bd_b = consts.tile([P, J], FP32)
            nc.gpsimd.dma_start(out=bd_b, in_=bd_d[:].unsqueeze(0).partition_broadcast(P))
            ones = consts.tile([1, P], FP32)
            nc.gpsimd.memset(ones, 1.0)

            # v replicated across partitions, one tile per chunk so each
            # STT depends only on ITS chunk's writers (no false deps).
            v_cs = [
                consts.tile([P, CHUNKS[c]], FP32, name=f"vc{c}", tag=f"vc{c}")
                for c in range(n_ch)
            ]
            # chunk 0: stride-0 DMA broadcast (4 MiB of bus traffic) split
            # across both rings ahead of their phase-0 x tiles.  This costs
            # ~9us of bus time but keeps phase 0's DVE demand well under
            # its DMA time, which is what keeps the pipeline out of the
            # slot-gated regime (measured: SWDGE starts LATER than the HW
            # rings, so prefetching via gpsimd does not work).
            h = c0 // 2
            for r in range(2):
                rings[r].dma_start(
                    out=v_cs[0][:, r * h:(r + 1) * h],
                    in_=v_d[r * h:(r + 1) * h].unsqueeze(0).partition_broadcast(P),
                )

            def emit_vchunk(c: int):
                # chunk c (c>=1): on-chip replicate.  ones[1,P].T @ v
                # (K=1 so each output is a single product => exact copy);
                # PSUM->SBUF copies on the Activation engine.  vrow is
                # staged through SBUF in <=VR_PIECE sub-pieces.
                f, off = CHUNKS[c], offs[c]
                for sub in range(0, f, VR_PIECE):
                    fs = min(VR_PIECE, f - sub)
                    vr_t = vrp.tile([1, fs], FP32, name=f"vr{c}_{sub}", tag="vr")
                    nc.gpsimd.dma_start(
                        out=vr_t, in_=v_d[off + sub:off + sub + fs].unsqueeze(0)
                    )
                    for k in range(fs // MM):
                        pt = psum.tile([P, MM], FP32, name=f"pt{c}_{sub}_{k}", tag="pt")
                        nc.tensor.matmul(
                            pt, ones, vr_t[:, k * MM:(k + 1) * MM],
                            start=True, stop=True,
                        )
                        nc.scalar.copy(
                            out=v_cs[c][:, sub + k * MM:sub + (k + 1) * MM], in_=pt
                        )

            accs = [
                accp.tile([P, n_ch], FP32, name=f"acc{bb}", tag=f"acc{bb}")
                for bb in range(N_BB)
            ]

            for c in range(n_ch):
                f, off = CHUNKS[c], offs[c]
                # x DMAs for this phase first so both rings stay fed...
                xts = []
                for bb in range(N_BB):
                    x_t = xp.tile([P, f], FP32, name=f"x{c}_{bb}", tag="x")
                    rings[bb % 2].dma_start(out=x_t, in_=x_ds[bb][:, off:off + f])
                    xts.append(x_t)
                # ...then the NEXT phase's v replication (Act engine work
                # lands between this phase's and next phase's enqueues).
                if c + 1 < n_ch:
                    emit_vchunk(c + 1)
                for bb in range(N_BB):
                    # x_t *= v (in place); acc[:, c] = sum over free dim.
                    # The DVE runs ONLY these streaming passes.
                    nc.vector.scalar_tensor_tensor(
                        out=xts[bb],
                        in0=xts[bb],
                        scalar=1.0,
                        in1=v_cs[c],
                        op0=mybir.AluOpType.mult,
                        op1=mybir.AluOpType.mult,
                        accum_out=accs[bb][:, c:c + 1],
                    )
                    if c == n_ch - 1:
                        # epilogue off the DVE: Act reduces acc via
                        # activation's accumulator and forms t*wd via the
                        # per-partition scale operand; Pool adds bd_eff and
                        # writes out.
                        tacc = accp.tile(
                            [P, accs[bb].shape[1]], FP32, name=f"ta{bb}", tag="ta"
                        )
                        t = accp.tile([P, 1], FP32, name=f"t{bb}", tag="t")
                        nc.scalar.activation(
                            out=tacc, in_=accs[bb],
                            func=mybir.ActivationFunctionType.Copy,
                            bias=0.0, scale=1.0, accum_out=t,
                        )
                        o1 = outp.tile([P, J], FP32, name=f"o1_{bb}", tag="o1")
                        nc.scalar.activation(
                            out=o1, in_=wd_b,
                            func=mybir.ActivationFunctionType.Copy,
                            bias=0.0, scale=t,
                        )
                        o_t = outp.tile([P, J], FP32, name=f"o{bb}", tag="o")
                        if bb == N_BB - 1:
                            # last block is the kernel tail: Pool's
                            # tensor_add is ~2.5us, DVE's is ~0.3us, and
                            # the SP ring enqueues faster than SWDGE.
                            nc.vector.tensor_add(out=o_t, in0=o1, in1=bd_b)
                            nc.sync.dma_start(
                                out=out_d[bb * P:(bb + 1) * P, :], in_=o_t
                            )
                        else:
                            nc.gpsimd.tensor_add(out=o_t, in0=o1, in1=bd_b)
                            nc.gpsimd.dma_start(
                                out=out_d[bb * P:(bb + 1) * P, :], in_=o_t
                            )
    install_legalizer(nc)
    return nc


_module_cache: dict = {}


def get_module() -> bass.Bass:
    if "nc" not in _module_cache:
        _module_cache["nc"] = build_module()
    return _module_cache["nc"]


def make_in_maps(inputs: dict) -> list[dict]:
    """Shard the full inputs into one input map per core (pure data parallel
    on the batch dim; tiny weights replicated)."""
    x = np.ascontiguousarray(np.asarray(inputs["x"], dtype=np.float32))
    w1 = np.asarray(inputs["w1"], dtype=np.float32)
    v = np.ascontiguousarray(w1[0, :, 0])
    s0 = float(sum(
        np.asarray(inputs[k], np.float32).reshape(-1)[0]
        for k in ("b1", "b2", "b3", "b4", "b5")
    ))
    wd_row = np.ascontiguousarray(np.asarray(inputs["wd"], np.float32)[0, :])
    bd = np.asarray(inputs["bd"], np.float32).reshape(-1)
    bd_eff = np.ascontiguousarray((s0 * wd_row + bd).astype(np.float32))

    maps = []
    for c in range(N_CORES):
        m = {"v": v, "wdrow": wd_row, "bdeff": bd_eff}
        base = c * B_CORE
        for bb in range(B_CORE // P):
            m[f"x{bb}"] = np.ascontiguousarray(x[base + bb * P:base + (bb + 1) * P])
        maps.append(m)
    return maps


def kernel(**inputs) -> np.ndarray:
    nc = get_module()
    in_maps = make_in_maps(inputs)
    res = run_bass_kernel_spmd(nc, in_maps, core_ids=list(range(N_CORES)))
    return np.concatenate([r["out"] for r in res.results], axis=0)
